# revision 2
# baseline (speedup 1.0000x reference)
"""Trainium2 Bass kernel for batched single-head attention with QKV projections.

Reference computation (B=4, Lq=Lk=2048, Dm=1024, Dk=Dv=128):
    q = Q @ WQ + bQ ; k = K @ WK + bK ; v = V @ WV + bV
    out = softmax(q k^T / sqrt(Dk)) v

Sharding: 8 cores; core c handles batch b=c//2, query half h=c%2
(1024 queries per core). K/V for the batch are replicated across the
pair. All device inputs are pre-transposed on the host to [dm, seq]
layout and cast to bf16 so every matmul contracts along the partition
dim at 1 cycle/row.

Softmax is computed without max-subtraction (scores ~ N(0,1), max over
8M samples ~ 5.7 sigma -> exp <= ~300, safely in range): scoresT[k,q]
tiles come out of the PE, ScalarE applies exp(scale*x) straight from
PSUM into bf16 SBUF tiles, and the denominator falls out of the AV
matmul via a ones-column planted in v by a rank-1 bias matmul.
"""

import os
import sys

sys.path.insert(0, "/opt/trn_rl_repo")

import numpy as np
import ml_dtypes

import concourse.bass as bass
import concourse.bacc as bacc
import concourse.tile as tile
import concourse.mybir as mybir
from concourse.bass_utils import run_bass_kernel_spmd

BF16 = ml_dtypes.bfloat16

B, LQ, LK, DM, DK, DV = 4, 2048, 2048, 1024, 128, 128
N_CORES = 8
LQ_C = LQ // 2          # queries per core
N_DM = DM // 128        # dm blocks
N_KB = LK // 128        # k blocks
N_QB = LQ_C // 128      # q blocks per core
SCALE = 1.0 / float(np.sqrt(DK))

_CACHED_NC = None
LAST_EXEC_NS = None


def _build():
    dt = mybir.dt
    nc = bacc.Bacc("TRN2", target_bir_lowering=False, debug=False,
                   num_devices=N_CORES)

    qt_d = nc.dram_tensor("qt", [DM, LQ_C], dt.bfloat16, kind="ExternalInput")
    kt_d = nc.dram_tensor("kt", [DM, LK], dt.bfloat16, kind="ExternalInput")
    vt_d = nc.dram_tensor("vt", [DM, LK], dt.bfloat16, kind="ExternalInput")
    wq_d = nc.dram_tensor("wq", [DM, DK], dt.bfloat16, kind="ExternalInput")
    wk_d = nc.dram_tensor("wk", [DM, DK], dt.bfloat16, kind="ExternalInput")
    wv_d = nc.dram_tensor("wv", [DM, DV], dt.bfloat16, kind="ExternalInput")
    bq_d = nc.dram_tensor("bq", [DK, 1], dt.float32, kind="ExternalInput")
    bk_d = nc.dram_tensor("bk", [DK, 1], dt.float32, kind="ExternalInput")
    bva_d = nc.dram_tensor("bvaug", [1, DV + 1], dt.bfloat16, kind="ExternalInput")
    out_d = nc.dram_tensor("out", [LQ_C, DV], dt.float32, kind="ExternalOutput")

    with tile.TileContext(nc) as tc:
        with tc.tile_pool(name="sb", bufs=1) as sb:
            # --- resident SBUF tensors ---
            wq = sb.tile([128, N_DM, DK], dt.bfloat16)
            wk = sb.tile([128, N_DM, DK], dt.bfloat16)
            wv = sb.tile([128, N_DM, DV], dt.bfloat16)
            bq = sb.tile([DK, 1], dt.float32)
            bk = sb.tile([DK, 1], dt.float32)
            bva = sb.tile([1, DV + 1], dt.bfloat16)
            ones = sb.tile([1, 128], dt.bfloat16)
            qt_sb = sb.tile([128, N_DM, LQ_C], dt.bfloat16)
            kt_sb = sb.tile([128, N_DM, LK], dt.bfloat16)
            vt_sb = sb.tile([128, N_DM, LK], dt.bfloat16)
            qT = sb.tile([DK, LQ_C], dt.bfloat16)     # projected q, [dk, lq]
            kT = sb.tile([DK, LK], dt.bfloat16)       # projected k, [dk, lk]
            v_sb = sb.tile([128, N_KB, DV + 1], dt.bfloat16)  # [k, dv+1]
            pT = sb.tile([128, N_KB, 2, 512], dt.bfloat16)    # exp scores [k, q]
            out_sb = sb.tile([128, N_QB, DV], dt.float32)
            recip = sb.tile([128, N_QB, 1], dt.float32)

            nc.sync.dma_start(bq[:], bq_d.ap())
            nc.sync.dma_start(bk[:], bk_d.ap())
            nc.sync.dma_start(bva[:], bva_d.ap())
            nc.vector.memset(ones[:], 1.0)
            for i in range(N_DM):
                s = slice(i * 128, (i + 1) * 128)
                nc.sync.dma_start(wq[:, i, :], wq_d.ap()[s, :])
                nc.sync.dma_start(wk[:, i, :], wk_d.ap()[s, :])
                nc.sync.dma_start(wv[:, i, :], wv_d.ap()[s, :])
            for i in range(N_DM):
                s = slice(i * 128, (i + 1) * 128)
                nc.sync.dma_start(qt_sb[:, i, :], qt_d.ap()[s, :])
                nc.sync.dma_start(kt_sb[:, i, :], kt_d.ap()[s, :])
            for i in range(N_DM):
                s = slice(i * 128, (i + 1) * 128)
                nc.sync.dma_start(vt_sb[:, i, :], vt_d.ap()[s, :])

            # --- phase A: q/k projections (accumulate over dm blocks) ---
            with tc.tile_pool(name="ps_a", bufs=6, space="PSUM") as ps_a:
                psq = [ps_a.tile([128, 512], dt.float32, tag="ps_a",
                                     name=f"psq{j}") for j in range(2)]
                for i in range(N_DM):
                    for nt in range(2):
                        nc.tensor.matmul(
                            psq[nt][:], wq[:, i, :],
                            qt_sb[:, i, nt * 512:(nt + 1) * 512],
                            start=(i == 0), stop=(i == N_DM - 1))
                for nt in range(2):
                    nc.vector.tensor_scalar_add(
                        qT[:, nt * 512:(nt + 1) * 512], psq[nt][:], bq[:])

                psk = [ps_a.tile([128, 512], dt.float32, tag="ps_a",
                                     name=f"psk{j}") for j in range(4)]
                for nt in range(4):
                    for i in range(N_DM):
                        nc.tensor.matmul(
                            psk[nt][:], wk[:, i, :],
                            kt_sb[:, i, nt * 512:(nt + 1) * 512],
                            start=(i == 0), stop=(i == N_DM - 1))
                    nc.vector.tensor_scalar_add(
                        kT[:, nt * 512:(nt + 1) * 512], psk[nt][:], bk[:])

            # --- phase B: scores+exp, v projection, AV ---
            with tc.tile_pool(name="ps_s", bufs=2, space="PSUM") as ps_s, \
                 tc.tile_pool(name="ps_v", bufs=2, space="PSUM") as ps_v, \
                 tc.tile_pool(name="ps_o", bufs=2, space="PSUM") as ps_o:

                for kb in range(N_KB):
                    pss = ps_s.tile([128, 2, 512], dt.float32)
                    for nt in range(2):
                        nc.tensor.matmul(
                            pss[:, nt, :], kT[:, kb * 128:(kb + 1) * 128],
                            qT[:, nt * 512:(nt + 1) * 512],
                            start=True, stop=True)
                    nc.scalar.activation(
                        pT[:, kb, :, :], pss[:, :, :],
                        mybir.ActivationFunctionType.Exp, scale=SCALE)

                    # v projection for this k block (interleaves with scores)
                    psv = ps_v.tile([128, DV + 1], dt.float32)
                    nc.tensor.matmul(psv[:], ones[:1, :], bva[:1, :],
                                     start=True, stop=False)
                    for i in range(N_DM):
                        nc.tensor.matmul(
                            psv[:, 0:DV],
                            vt_sb[:, i, kb * 128:(kb + 1) * 128],
                            wv[:, i, :],
                            start=False, stop=(i == N_DM - 1))
                    nc.vector.tensor_copy(v_sb[:, kb, :], psv[:])

                for qb in range(N_QB):
                    pso = ps_o.tile([128, DV + 1], dt.float32)
                    for kb in range(N_KB):
                        nc.tensor.matmul(
                            pso[:],
                            pT[:, kb, qb // 4, (qb % 4) * 128:(qb % 4 + 1) * 128],
                            v_sb[:, kb, :],
                            start=(kb == 0), stop=(kb == N_KB - 1))
                    nc.vector.reciprocal(recip[:, qb, :], pso[:, DV:DV + 1])
                    nc.vector.tensor_scalar_mul(
                        out_sb[:, qb, :], pso[:, 0:DV], recip[:, qb, :])
                    nc.sync.dma_start(
                        out_d.ap()[qb * 128:(qb + 1) * 128, :],
                        out_sb[:, qb, :])

    nc.compile()
    return nc


def kernel(**inputs):
    global _CACHED_NC, LAST_EXEC_NS
    Q = np.asarray(inputs["Q"], dtype=np.float32)
    K = np.asarray(inputs["K"], dtype=np.float32)
    V = np.asarray(inputs["V"], dtype=np.float32)
    WQ = np.asarray(inputs["WQ"], dtype=np.float32)
    bQ = np.asarray(inputs["bQ"], dtype=np.float32)
    WK = np.asarray(inputs["WK"], dtype=np.float32)
    bK = np.asarray(inputs["bK"], dtype=np.float32)
    WV = np.asarray(inputs["WV"], dtype=np.float32)
    bV = np.asarray(inputs["bV"], dtype=np.float32)

    if _CACHED_NC is None:
        _CACHED_NC = _build()
    nc = _CACHED_NC

    wq = np.ascontiguousarray(WQ).astype(BF16)
    wk = np.ascontiguousarray(WK).astype(BF16)
    wv = np.ascontiguousarray(WV).astype(BF16)
    bq = bQ.reshape(DK, 1).astype(np.float32)
    bk = bK.reshape(DK, 1).astype(np.float32)
    bva = np.concatenate([bV, np.ones(1, np.float32)]).reshape(1, DV + 1).astype(BF16)

    kt_b = [np.ascontiguousarray(K[b].T).astype(BF16) for b in range(B)]
    vt_b = [np.ascontiguousarray(V[b].T).astype(BF16) for b in range(B)]

    in_maps = []
    for c in range(N_CORES):
        b, h = c // 2, c % 2
        qt = np.ascontiguousarray(Q[b, h * LQ_C:(h + 1) * LQ_C, :].T).astype(BF16)
        in_maps.append({
            "qt": qt, "kt": kt_b[b], "vt": vt_b[b],
            "wq": wq, "wk": wk, "wv": wv,
            "bq": bq, "bk": bk, "bvaug": bva,
        })

    trace = bool(os.environ.get("KERNEL_TRACE"))
    if trace:
        import axon_profile_shim  # noqa: F401

    res = run_bass_kernel_spmd(nc, in_maps, core_ids=list(range(N_CORES)),
                               trace=trace)
    LAST_EXEC_NS = res.exec_time_ns

    out = np.empty((B, LQ, DV), np.float32)
    for c in range(N_CORES):
        b, h = c // 2, c % 2
        out[b, h * LQ_C:(h + 1) * LQ_C, :] = res.results[c]["out"]
    return out


# revision 3
# speedup vs baseline: 1.0956x; 1.0956x over previous
"""Trainium2 Bass kernel for batched single-head attention with QKV projections.

Reference computation (B=4, Lq=Lk=2048, Dm=1024, Dk=Dv=128):
    q = Q @ WQ + bQ ; k = K @ WK + bK ; v = V @ WV + bV
    out = softmax(q k^T / sqrt(Dk)) v

Sharding: 8 cores; core c handles batch b=c//2, query half h=c%2
(1024 queries per core). K/V for the batch are replicated across the
pair. All device inputs are pre-transposed on the host to [dm, seq]
layout and cast to bf16 so every matmul contracts along the partition
dim at 1 cycle/row.

Softmax is computed without max-subtraction (scores ~ N(0,1), max over
8M samples ~ 5.7 sigma -> exp <= ~300, safely in range): scoresT[k,q]
tiles come out of the PE, ScalarE applies exp(scale*x) straight from
PSUM into bf16 SBUF tiles, and the denominator falls out of the AV
matmul via a ones-column planted in v by a rank-1 bias matmul.
"""

import os
import sys

sys.path.insert(0, "/opt/trn_rl_repo")

import numpy as np
import ml_dtypes

import concourse.bass as bass
import concourse.bacc as bacc
import concourse.tile as tile
import concourse.mybir as mybir
from concourse.bass_utils import run_bass_kernel_spmd

BF16 = ml_dtypes.bfloat16

B, LQ, LK, DM, DK, DV = 4, 2048, 2048, 1024, 128, 128
N_CORES = 8
LQ_C = LQ // 2          # queries per core
N_DM = DM // 128        # dm blocks
N_KB = LK // 128        # k blocks
N_QB = LQ_C // 128      # q blocks per core
SCALE = 1.0 / float(np.sqrt(DK))

_CACHED_NC = None
LAST_EXEC_NS = None


def _build():
    dt = mybir.dt
    nc = bacc.Bacc("TRN2", target_bir_lowering=False, debug=False,
                   num_devices=N_CORES)

    qt_d = nc.dram_tensor("qt", [DM, LQ_C], dt.bfloat16, kind="ExternalInput")
    kt_d = nc.dram_tensor("kt", [DM, LK], dt.bfloat16, kind="ExternalInput")
    vt_d = nc.dram_tensor("vt", [DM, LK], dt.bfloat16, kind="ExternalInput")
    w_d = nc.dram_tensor("w", [DM, 3, 128], dt.bfloat16, kind="ExternalInput")
    b2_d = nc.dram_tensor("b2", [DK, 2], dt.float32, kind="ExternalInput")
    bva_d = nc.dram_tensor("bvaug", [1, DV + 1], dt.bfloat16, kind="ExternalInput")
    out_d = nc.dram_tensor("out", [LQ_C, DV], dt.float32, kind="ExternalOutput")

    with tile.TileContext(nc) as tc:
        with tc.tile_pool(name="sb", bufs=1) as sb:
            # --- resident SBUF tensors ---
            w_sb = sb.tile([128, N_DM, 3, 128], dt.bfloat16)
            b2 = sb.tile([DK, 2], dt.float32)
            bva = sb.tile([1, DV + 1], dt.bfloat16)
            ones = sb.tile([1, 128], dt.bfloat16)
            qt_sb = sb.tile([128, N_DM, LQ_C], dt.bfloat16)
            kt_sb = sb.tile([128, N_DM, LK], dt.bfloat16)
            vt_sb = sb.tile([128, N_DM, LK], dt.bfloat16)
            qT = sb.tile([DK, LQ_C], dt.bfloat16)     # projected q, [dk, lq]
            kT = sb.tile([DK, LK], dt.bfloat16)       # projected k, [dk, lk]
            v_sb = sb.tile([128, N_KB, DV + 1], dt.bfloat16)  # [k, dv+1]
            pT = sb.tile([128, N_KB, 2, 512], dt.bfloat16)    # exp scores [k, q]
            out_sb = sb.tile([128, N_QB, DV], dt.float32)
            recip = sb.tile([128, N_QB, 1], dt.float32)

            nc.sync.dma_start(w_sb[:], w_d.ap().rearrange("(i p) c j -> p i c j", p=128))
            nc.sync.dma_start(b2[:], b2_d.ap())
            nc.sync.dma_start(bva[:], bva_d.ap())
            nc.vector.memset(ones[:], 1.0)
            qt_r = qt_d.ap().rearrange("(i p) j -> p i j", p=128)
            kt_r = kt_d.ap().rearrange("(i p) j -> p i j", p=128)
            vt_r = vt_d.ap().rearrange("(i p) j -> p i j", p=128)
            for half in range(2):
                hs = slice(half * 4, (half + 1) * 4)
                nc.sync.dma_start(qt_sb[:, hs, :], qt_r[:, hs, :])
                nc.sync.dma_start(kt_sb[:, hs, :], kt_r[:, hs, :])
            for half in range(2):
                hs = slice(half * 4, (half + 1) * 4)
                nc.scalar.dma_start(vt_sb[:, hs, :], vt_r[:, hs, :])

            # --- phase A: q/k projections (accumulate over dm blocks) ---
            with tc.tile_pool(name="ps_a", bufs=6, space="PSUM") as ps_a:
                psq = [ps_a.tile([128, 512], dt.float32, tag="ps_a",
                                     name=f"psq{j}") for j in range(2)]
                for i in range(N_DM):
                    for nt in range(2):
                        nc.tensor.matmul(
                            psq[nt][:], w_sb[:, i, 0, :],
                            qt_sb[:, i, nt * 512:(nt + 1) * 512],
                            start=(i == 0), stop=(i == N_DM - 1))
                for nt in range(2):
                    nc.vector.tensor_scalar_add(
                        qT[:, nt * 512:(nt + 1) * 512], psq[nt][:], b2[:, 0:1])

                psk = [ps_a.tile([128, 512], dt.float32, tag="ps_a",
                                     name=f"psk{j}") for j in range(4)]
                for nt in range(4):
                    for i in range(N_DM):
                        nc.tensor.matmul(
                            psk[nt][:], w_sb[:, i, 1, :],
                            kt_sb[:, i, nt * 512:(nt + 1) * 512],
                            start=(i == 0), stop=(i == N_DM - 1))
                    nc.vector.tensor_scalar_add(
                        kT[:, nt * 512:(nt + 1) * 512], psk[nt][:], b2[:, 1:2])

            # --- phase B: scores+exp, v projection, AV ---
            with tc.tile_pool(name="ps_s", bufs=2, space="PSUM") as ps_s, \
                 tc.tile_pool(name="ps_v", bufs=2, space="PSUM") as ps_v, \
                 tc.tile_pool(name="ps_o", bufs=2, space="PSUM") as ps_o:

                for kb in range(N_KB):
                    pss = ps_s.tile([128, 2, 512], dt.float32)
                    for nt in range(2):
                        nc.tensor.matmul(
                            pss[:, nt, :], kT[:, kb * 128:(kb + 1) * 128],
                            qT[:, nt * 512:(nt + 1) * 512],
                            start=True, stop=True)
                    nc.scalar.activation(
                        pT[:, kb, :, :], pss[:, :, :],
                        mybir.ActivationFunctionType.Exp, scale=SCALE)

                    # v projection for this k block (interleaves with scores)
                    psv = ps_v.tile([128, DV + 1], dt.float32)
                    nc.tensor.matmul(psv[:], ones[:1, :], bva[:1, :],
                                     start=True, stop=False)
                    for i in range(N_DM):
                        nc.tensor.matmul(
                            psv[:, 0:DV],
                            vt_sb[:, i, kb * 128:(kb + 1) * 128],
                            w_sb[:, i, 2, :],
                            start=False, stop=(i == N_DM - 1))
                    nc.vector.tensor_copy(v_sb[:, kb, :], psv[:])

                out_r = out_d.ap().rearrange("(i p) j -> p i j", p=128)
                for qb in range(N_QB):
                    pso = ps_o.tile([128, DV + 1], dt.float32)
                    for kb in range(N_KB):
                        nc.tensor.matmul(
                            pso[:],
                            pT[:, kb, qb // 4, (qb % 4) * 128:(qb % 4 + 1) * 128],
                            v_sb[:, kb, :],
                            start=(kb == 0), stop=(kb == N_KB - 1))
                    nc.vector.reciprocal(recip[:, qb, :], pso[:, DV:DV + 1])
                    nc.vector.tensor_scalar_mul(
                        out_sb[:, qb, :], pso[:, 0:DV], recip[:, qb, :])
                    if qb % 2 == 1:
                        nc.gpsimd.dma_start(
                            out_r[:, qb - 1:qb + 1, :],
                            out_sb[:, qb - 1:qb + 1, :])

    nc.compile()
    return nc


def kernel(**inputs):
    global _CACHED_NC, LAST_EXEC_NS
    Q = np.asarray(inputs["Q"], dtype=np.float32)
    K = np.asarray(inputs["K"], dtype=np.float32)
    V = np.asarray(inputs["V"], dtype=np.float32)
    WQ = np.asarray(inputs["WQ"], dtype=np.float32)
    bQ = np.asarray(inputs["bQ"], dtype=np.float32)
    WK = np.asarray(inputs["WK"], dtype=np.float32)
    bK = np.asarray(inputs["bK"], dtype=np.float32)
    WV = np.asarray(inputs["WV"], dtype=np.float32)
    bV = np.asarray(inputs["bV"], dtype=np.float32)

    if _CACHED_NC is None:
        _CACHED_NC = _build()
    nc = _CACHED_NC

    w = np.ascontiguousarray(
        np.stack([WQ, WK, WV], axis=1)).astype(BF16)  # [DM, 3, 128]
    b2 = np.ascontiguousarray(
        np.stack([bQ, bK], axis=1)).astype(np.float32)  # [DK, 2]
    bva = np.concatenate([bV, np.ones(1, np.float32)]).reshape(1, DV + 1).astype(BF16)

    kt_b = [np.ascontiguousarray(K[b].T).astype(BF16) for b in range(B)]
    vt_b = [np.ascontiguousarray(V[b].T).astype(BF16) for b in range(B)]

    in_maps = []
    for c in range(N_CORES):
        b, h = c // 2, c % 2
        qt = np.ascontiguousarray(Q[b, h * LQ_C:(h + 1) * LQ_C, :].T).astype(BF16)
        in_maps.append({
            "qt": qt, "kt": kt_b[b], "vt": vt_b[b],
            "w": w, "b2": b2, "bvaug": bva,
        })

    trace = bool(os.environ.get("KERNEL_TRACE"))
    if trace:
        import axon_profile_shim  # noqa: F401

    res = run_bass_kernel_spmd(nc, in_maps, core_ids=list(range(N_CORES)),
                               trace=trace)
    LAST_EXEC_NS = res.exec_time_ns

    out = np.empty((B, LQ, DV), np.float32)
    for c in range(N_CORES):
        b, h = c // 2, c % 2
        out[b, h * LQ_C:(h + 1) * LQ_C, :] = res.results[c]["out"]
    return out
